# revision 1
# baseline (speedup 1.0000x reference)
"""Multi-head attention (b=4, n=2048, dim=512, h=8, dh=64) on 8 trn2 cores.

Sharding: core c handles batch b=c//2 and query rows
[half*1024, (half+1)*1024) with half=c%2. K/V (from x_prev) are computed
redundantly on both of a batch's cores (cheap vs. attention). No collectives.

Per-core kernel (bf16 operands, fp32 PSUM accumulation):
  QT[inner, nq]  = w_q-tiles  (lhsT) @ x^T          (q in transposed layout)
  KT[inner, nk]  = w_kv-tiles (lhsT) @ x_prev^T
  V [nk, inner]  = x_prev^T-tiles (lhsT) @ w_kv[:, v]  (natural layout,
                                                        + ones column/head)
  ST[j, i]       = KT_h-tile (lhsT, K=dh=64) @ QT_h  (scores transposed;
                   two heads row-tiled in the PE at partitions 0/64)
  PT             = exp(ST * scale)    (no max subtraction: |s*scale| < ~8)
  PV[dh+1, i]    = sum_j V_h|ones (lhsT) @ PT_h      (row dh = sum_j P = l)
  l -> (dma to partitions 0/1) -> r = 1/l -> rb_h = ones-matmul broadcast
  AOT[0:64, h]   = PV[0:dh] * rb_h                   (per-head, partitions 0-63)
  out[i, d]      = sum_h AOT_h-tiles (lhsT, K=64) @ w_out_h + ones @ b_out
"""

import numpy as np
import ml_dtypes

B, N, DIM = 4, 2048, 512
H, DH, INNER = 8, 64, 512
NCORES = 8

_BUILT = None


def build_module(dim=DIM, h=H, nq=N // 2, nk=N, compile_module=True, reps=1,
                 stub=frozenset()):
    """Build the per-core attention module. dim % 128 == 0, h % 2 == 0,
    nq % 512 == 0, nk % 128 == 0. reps>1 repeats the whole compute body
    (timing calibration only)."""
    import concourse.mybir as mybir
    import concourse.tile as tile
    from concourse import bacc

    CDT = mybir.dt.bfloat16
    FDT = mybir.dt.float32
    Exp = mybir.ActivationFunctionType.Exp

    inner = h * DH
    nkt = dim // 128          # contraction tiles for projections
    npr = h // 2              # head pairs (= inner // 128 slices of KT/QT)
    nj = nk // 128            # key tiles
    nqc = nq // 512           # query chunks
    VW = DH + 1               # 65: per-head v columns + ones column
    scale = DH ** -0.5

    nc = bacc.Bacc("TRN2", target_bir_lowering=False, debug=False,
                   num_devices=NCORES)

    xt_d = nc.declare_dram_parameter("xt", [dim, nq], CDT, isOutput=False)
    xpt_d = nc.declare_dram_parameter("xpt", [dim, nk], CDT, isOutput=False)
    wq_d = nc.declare_dram_parameter("wq", [dim, inner], CDT, isOutput=False)
    wkv_d = nc.declare_dram_parameter("wkv", [dim, 2 * inner], CDT,
                                      isOutput=False)
    # w_out pre-arranged on host: wout2[d, h, :] = w_out[h*64+d, :]
    wout_d = nc.declare_dram_parameter("wout", [DH, h, dim], CDT,
                                       isOutput=False)
    bout_d = nc.declare_dram_parameter("bout", [1, dim], CDT, isOutput=False)
    out_d = nc.declare_dram_parameter("out", [nq, dim], FDT, isOutput=True)
    # DRAM bounce rows for the 1/l partition-broadcast (SBUF APs cannot
    # have a zero-step partition dim; DRAM APs can).
    rsc_d = nc.dram_tensor("rscratch", [h * nq // 512, 512], FDT)

    import contextlib
    with tile.TileContext(nc) as tc, contextlib.ExitStack() as stack:
        consts = stack.enter_context(tc.tile_pool(name="consts", bufs=1))
        acts = stack.enter_context(tc.tile_pool(name="acts", bufs=1))

        # ---- constants / weights ----------------------------------------
        wq_sb = consts.tile([128, nkt, inner], CDT)
        wkv_sb = consts.tile([128, nkt, 2 * inner], CDT)
        wout_sb = consts.tile([DH, h, dim], CDT)
        bout_sb = consts.tile([1, dim], CDT)
        ones_sb = consts.tile([1, 128], CDT)

        for k in range(nkt):
            nc.sync.dma_start(
                out=wq_sb[:, k, :],
                in_=wq_d.ap().rearrange("(t p) o -> p t o", p=128)[:, k, :])
            nc.sync.dma_start(
                out=wkv_sb[:, k, :],
                in_=wkv_d.ap().rearrange("(t p) o -> p t o", p=128)[:, k, :])
        nc.sync.dma_start(out=wout_sb[:, :, :], in_=wout_d.ap())
        nc.sync.dma_start(out=bout_sb[:, :], in_=bout_d.ap())
        nc.vector.memset(ones_sb[:, :], 1.0)

        # ---- activations -------------------------------------------------
        xt_sb = acts.tile([128, nkt, nq], CDT)
        xpt_sb = acts.tile([128, nkt, nk], CDT)
        for k in range(nkt):
            nc.sync.dma_start(
                out=xt_sb[:, k, :],
                in_=xt_d.ap().rearrange("(t p) n -> p t n", p=128)[:, k, :])
            nc.sync.dma_start(
                out=xpt_sb[:, k, :],
                in_=xpt_d.ap().rearrange("(t p) n -> p t n", p=128)[:, k, :])

        qt_sb = acts.tile([128, npr, nq], CDT)    # [inner-slice, nq]
        kt_sb = acts.tile([128, npr, nk], CDT)    # [inner-slice, nk]
        v_sb = acts.tile([128, nj, h * VW], CDT)  # [key-tile, h*(dh+1)]
        aot_sb = acts.tile([DH, h, nq], CDT)      # [dh, head, nq]

        for hh in range(h):  # ones columns of V
            nc.vector.memset(v_sb[:, :, hh * VW + DH:hh * VW + DH + 1], 1.0)

        for _rep in range(reps):
            # KT/QT slice 0 first so attention can start early; V interleaved.
            kw = min(512, nk)   # kt projection chunk width
            order = []
            for s in range(npr):
                for c in range(nk // kw):
                    order.append(("kt", s, c))
                for c in range(nqc):
                    order.append(("qt", s, c))
                if s == 0:
                    for j in range(nj):
                        order.append(("v", j, 0))
            proj_scope = tc.tile_pool(name="proj_ps", bufs=4, space="PSUM")
            proj_ps = proj_scope.__enter__()
            for kind, a, c in order:
                ps = proj_ps.tile([128, 512], FDT, tag="mm")
                if kind == "kt":
                    for k in range(nkt):
                        nc.tensor.matmul(
                            ps[:, 0:kw], lhsT=wkv_sb[:, k, a * 128:(a + 1) * 128],
                            rhs=xpt_sb[:, k, c * kw:(c + 1) * kw],
                            start=(k == 0), stop=(k == nkt - 1))
                    nc.vector.tensor_copy(
                        out=kt_sb[:, a, c * kw:(c + 1) * kw], in_=ps[:, 0:kw])
                elif kind == "qt":
                    for k in range(nkt):
                        nc.tensor.matmul(
                            ps[:, :], lhsT=wq_sb[:, k, a * 128:(a + 1) * 128],
                            rhs=xt_sb[:, k, c * 512:(c + 1) * 512],
                            start=(k == 0), stop=(k == nkt - 1))
                    nc.vector.tensor_copy(
                        out=qt_sb[:, a, c * 512:(c + 1) * 512], in_=ps[:, :])
                else:  # v: natural layout, lhsT = xpt token-tile
                    for k in range(nkt):
                        nc.tensor.matmul(
                            ps[:, 0:inner],
                            lhsT=xpt_sb[:, k, a * 128:(a + 1) * 128],
                            rhs=wkv_sb[:, k, inner:2 * inner],
                            start=(k == 0), stop=(k == nkt - 1))
                    nc.vector.tensor_copy(
                        out=v_sb[:, a, :].rearrange(
                            "p (g x) -> p g x", x=VW)[:, :, 0:DH],
                        in_=ps[:, 0:inner].rearrange("p (g x) -> p g x", x=DH))

            proj_scope.__exit__(None, None, None)

            # ---- attention ---------------------------------------------------
            attn_stack = contextlib.ExitStack()
            st_ps = attn_stack.enter_context(
                tc.tile_pool(name="st_ps", bufs=2, space="PSUM"))
            acc_ps = attn_stack.enter_context(
                tc.tile_pool(name="acc_ps", bufs=4, space="PSUM"))
            pt_pool = attn_stack.enter_context(tc.tile_pool(name="pt", bufs=2))
            lr_pool = attn_stack.enter_context(tc.tile_pool(name="lr", bufs=3))

            for c in range(nqc):          # query chunk of 512
                for p in range(npr):      # head pair (2p, 2p+1)
                    h0, h1 = 2 * p, 2 * p + 1
                    pt = pt_pool.tile([128, nj, 1024], CDT, tag="pt")
                    for j in range(nj):
                        st = st_ps.tile([128, 1024], FDT, tag="st")
                        nc.tensor.matmul(
                            st[:, 0:512],
                            lhsT=kt_sb[0:64, p, j * 128:(j + 1) * 128],
                            rhs=qt_sb[0:64, p, c * 512:(c + 1) * 512],
                            start=True, stop=True)
                        nc.tensor.matmul(
                            st[:, 512:1024],
                            lhsT=kt_sb[64:128, p, j * 128:(j + 1) * 128],
                            rhs=qt_sb[64:128, p, c * 512:(c + 1) * 512],
                            start=True, stop=True)
                        if "noexp" in stub:
                            nc.vector.tensor_copy(out=pt[:, j, :],
                                                  in_=st[:, :])
                        else:
                            nc.scalar.activation(out=pt[:, j, :], in_=st[:, :],
                                                 func=Exp, scale=scale)

                    pv0 = acc_ps.tile([128, 512], FDT, tag="acc")
                    pv1 = acc_ps.tile([128, 512], FDT, tag="acc")
                    for j in range(nj):
                        nc.tensor.matmul(
                            pv0[0:VW, :], lhsT=v_sb[:, j, h0 * VW:(h0 + 1) * VW],
                            rhs=pt[:, j, 0:512],
                            start=(j == 0), stop=(j == nj - 1))
                        nc.tensor.matmul(
                            pv1[0:VW, :], lhsT=v_sb[:, j, h1 * VW:(h1 + 1) * VW],
                            rhs=pt[:, j, 512:1024],
                            start=(j == 0), stop=(j == nj - 1))

                    if "nonorm" in stub:
                        nc.vector.tensor_copy(
                            out=aot_sb[:, h0, c * 512:(c + 1) * 512],
                            in_=pv0[0:DH, :])
                        nc.vector.tensor_copy(
                            out=aot_sb[:, h1, c * 512:(c + 1) * 512],
                            in_=pv1[0:DH, :])
                        continue
                    # softmax denominators: 1/l on lane DH, then a step-0
                    # partition DMA broadcasts it to 64 partitions in SBUF.
                    ra_sb = lr_pool.tile([DH + 1, 512], FDT, tag="ra")
                    rc_sb = lr_pool.tile([DH + 1, 512], FDT, tag="rc")
                    nc.vector.reciprocal(out=ra_sb[DH:DH + 1, :],
                                         in_=pv0[DH:DH + 1, :])
                    nc.vector.reciprocal(out=rc_sb[DH:DH + 1, :],
                                         in_=pv1[DH:DH + 1, :])
                    idx = (c * npr + p) * 2
                    nc.sync.dma_start(out=rsc_d.ap()[idx:idx + 1, :],
                                      in_=ra_sb[DH:DH + 1, :])
                    nc.sync.dma_start(out=rsc_d.ap()[idx + 1:idx + 2, :],
                                      in_=rc_sb[DH:DH + 1, :])
                    rb0 = lr_pool.tile([DH, 512], FDT, tag="rb0")
                    rb1 = lr_pool.tile([DH, 512], FDT, tag="rb1")
                    nc.gpsimd.dma_start(
                        out=rb0[:, :],
                        in_=rsc_d.ap()[idx:idx + 1, :].to_broadcast([DH, 512]))
                    nc.gpsimd.dma_start(
                        out=rb1[:, :],
                        in_=rsc_d.ap()[idx + 1:idx + 2, :].to_broadcast([DH, 512]))
                    nc.vector.tensor_mul(
                        aot_sb[:, h0, c * 512:(c + 1) * 512],
                        pv0[0:DH, :], rb0[:, :])
                    nc.vector.tensor_mul(
                        aot_sb[:, h1, c * 512:(c + 1) * 512],
                        pv1[0:DH, :], rb1[:, :])

                # ---- output projection for this chunk (4 row-tiles of 128) ---
                for t in range(4 * c, 4 * c + 4):
                    f = acc_ps.tile([128, 512], FDT, tag="acc")
                    for hh in range(h):
                        nc.tensor.matmul(
                            f[:, 0:dim],
                            lhsT=aot_sb[:, hh, t * 128:(t + 1) * 128],
                            rhs=wout_sb[:, hh, :],
                            start=(hh == 0), stop=False)
                    nc.tensor.matmul(f[:, 0:dim], lhsT=ones_sb[:, :],
                                     rhs=bout_sb[:, :], start=False, stop=True)
                    fo = lr_pool.tile([128, dim], FDT, tag="fo")
                    nc.vector.tensor_copy(out=fo[:, :], in_=f[:, 0:dim])
                    nc.sync.dma_start(
                        out=out_d.ap()[t * 128:(t + 1) * 128, :], in_=fo[:, :])
            attn_stack.close()

    if compile_module:
        nc.compile()
    return nc


def host_inputs(x, x_prev, w_q, w_kv, w_out, b_out, ncores=NCORES):
    """Shard + lay out the full inputs into per-core input maps."""
    bf16 = ml_dtypes.bfloat16
    b, n, dim = x.shape
    inner = w_q.shape[1]
    h = inner // DH
    nq = (b * n) // ncores
    halves = ncores // b
    wq = np.ascontiguousarray(w_q).astype(bf16)
    wkv = np.ascontiguousarray(w_kv).astype(bf16)
    wout = np.ascontiguousarray(
        w_out.reshape(h, DH, dim).transpose(1, 0, 2)).astype(bf16)
    bout = np.ascontiguousarray(b_out).reshape(1, dim).astype(bf16)
    in_maps = []
    for c in range(ncores):
        bb, half = c // halves, c % halves
        xt = np.ascontiguousarray(
            x[bb, half * nq:(half + 1) * nq, :].T).astype(bf16)
        xpt = np.ascontiguousarray(x_prev[bb].T).astype(bf16)
        in_maps.append(dict(xt=xt, xpt=xpt, wq=wq, wkv=wkv, wout=wout,
                            bout=bout))
    return in_maps


def _get_module():
    global _BUILT
    if _BUILT is None:
        _BUILT = build_module()
    return _BUILT


def kernel(x, x_prev, w_q, w_kv, w_out, b_out):
    from concourse.bass_utils import run_bass_kernel_spmd

    nc = _get_module()
    in_maps = host_inputs(x, x_prev, w_q, w_kv, w_out, b_out)
    res = run_bass_kernel_spmd(nc, in_maps, core_ids=list(range(NCORES)))

    nq = N // 2
    out = np.empty((B, N, DIM), np.float32)
    for c in range(NCORES):
        b, half = c // 2, c % 2
        out[b, half * nq:(half + 1) * nq, :] = res.results[c]["out"]
    return out



# revision 47
# speedup vs baseline: 9.3786x; 9.3786x over previous
"""Multi-head attention (b=4, n=2048, dim=512, h=8, dh=64) on 8 trn2 cores.

Sharding: core c handles batch b=c//2 and query rows
[half*1024, (half+1)*1024) with half=c%2. K/V (from x_prev) are computed
redundantly on both of a batch's cores (cheap vs. attention). No collectives.

Per-core kernel (bf16 operands, fp32 PSUM accumulation), fused single
pipeline: projections are interleaved INTO the attention groups (one group
per head-pair x query-chunk) so the Activation engine's exp stream
(~133us busy) hides under the PE stream (~155us) instead of idling during
a separate projection phase. The PE is p-state warmed with dummy matmuls
while the first input DMAs are in flight.

  QT[inner, nq]  = w_q-tiles  (lhsT) @ x^T
  KT[inner, nk]  = w_kv-tiles (lhsT) @ x_prev^T
  V [nk, inner]  = x_prev^T-tiles (lhsT) @ w_kv[:, v]   (+ ones col/head)
  per group (head-pair p, query-chunk c), per key-tile j:
    ST[j, i]     = KT_h-tile (lhsT, K=dh=64) @ QT_h   (2 heads at part 0/64)
    PT           = exp(ST * scale)      (no max subtraction: |s*scale| < ~8)
    PV[dh+1, i] += V_h|ones (lhsT) @ PT_h   (lagged `lag` tiles behind the
                   scores; row dh accumulates l = sum_j P)
  1/l (DVE) -> partition-broadcast via DRAM bounce (last group: via a K=1
  ones-matmul, which is lower latency but costs PE time)
  AOT2[0:64, p]  = PV0[0:dh] * rb0  (DVE)             (head-pair-packed AOT)
  AOT2[64:128,p] = PV1[0:dh] * rb1 -> SBUF-to-SBUF DMA partition shift
  out[i, d]      = sum_g AOT2_g-tiles (lhsT, K=128) @ wout2_g + ones @ b_out
  (the last chunk's out-proj runs pair-0..2 partial sums during the last
  group's 1/l chain, finishing with pair 3 + bias, PSUM copied out on
  DVE/Act alternately)
"""

import numpy as np
import ml_dtypes

B, N, DIM = 4, 2048, 512
H, DH, INNER = 8, 64, 512
NCORES = 8

_BUILT = None


def build_module(dim=DIM, h=H, nq=N // 2, nk=N, compile_module=True, reps=1,
                 stub=frozenset()):
    """Build the per-core attention module. dim % 128 == 0, h % 2 == 0,
    nq % 512 == 0, nk % 512 == 0. reps>1 repeats the whole compute body
    (timing calibration only)."""
    import concourse.mybir as mybir
    import concourse.tile as tile
    from concourse import bacc

    CDT = mybir.dt.bfloat16
    FDT = mybir.dt.float32
    Exp = mybir.ActivationFunctionType.Exp

    inner = h * DH
    nkt = dim // 128          # contraction tiles for projections
    npr = h // 2              # head pairs (= inner // 128 slices)
    nj = nk // 128            # key tiles
    nqc = nq // 512           # query chunks
    VW = DH + 1               # 65: per-head v columns + ones column
    scale = DH ** -0.5

    nc = bacc.Bacc("TRN2", target_bir_lowering=False, debug=False,
                   num_devices=NCORES)

    xt_d = nc.declare_dram_parameter("xt", [dim, nq], CDT, isOutput=False)
    xpt_d = nc.declare_dram_parameter("xpt", [dim, nk], CDT, isOutput=False)
    wq_d = nc.declare_dram_parameter("wq", [dim, inner], CDT, isOutput=False)
    wkv_d = nc.declare_dram_parameter("wkv", [dim, 2 * inner], CDT,
                                      isOutput=False)
    # w_out pre-arranged on host: wout2[r, g, :] = w_out[g*128 + r, :]
    wout_d = nc.declare_dram_parameter("wout", [128, npr, dim], CDT,
                                       isOutput=False)
    bout_d = nc.declare_dram_parameter("bout", [1, dim], CDT, isOutput=False)
    out_d = nc.declare_dram_parameter("out", [nq, dim], FDT, isOutput=True)
    # DRAM bounce rows for the 1/l partition-broadcast (SBUF APs cannot
    # have a zero-step partition dim; DRAM APs can).
    rsc_d = nc.dram_tensor("rscratch", [2 * h, 512], FDT)

    import contextlib
    with tile.TileContext(nc) as tc, contextlib.ExitStack() as stack:
        consts = stack.enter_context(tc.tile_pool(name="consts", bufs=1))
        acts = stack.enter_context(tc.tile_pool(name="acts", bufs=1))

        # ---- persistent tiles -------------------------------------------
        wq_sb = consts.tile([128, nkt, inner], CDT)
        wkv_sb = consts.tile([128, nkt, 2 * inner], CDT)
        wout_sb = consts.tile([128, npr, dim], CDT)
        bout_row = consts.tile([1, dim], CDT)
        ones_sb = consts.tile([1, 512], CDT)

        xt_sb = acts.tile([128, nkt, nq], CDT)
        xpt_sb = acts.tile([128, nkt, nk], CDT)
        qt_sb = acts.tile([128, npr, nq], CDT)    # [inner-slice, nq]
        kt_sb = acts.tile([128, npr, nk], CDT)    # [inner-slice, nk]
        v_sb = acts.tile([128, nj, h * VW], CDT)  # [key-tile, h*(dh+1)]
        aot2 = acts.tile([128, npr, nq], CDT)     # [pair-packed inner, nq]

        # ---- input DMA, two hw queues, priority-ordered ------------------
        wkv_r = wkv_d.ap().rearrange("(t p) o -> p t o", p=128)
        wq_r = wq_d.ap().rearrange("(t p) o -> p t o", p=128)
        xt_r = xt_d.ap().rearrange("(t p) n -> p t n", p=128)
        xpt_r = xpt_d.ap().rearrange("(t p) n -> p t n", p=128)
        # three parallel queues (SP + Act hwdge, gpsimd swdge), ordered so
        # the prologue (kt00, qt00) and first group (v tiles, kt0*) unblock
        # as early as possible.
        nc.sync.dma_start(out=xpt_sb[:, 0:2, 0:512], in_=xpt_r[:, 0:2, 0:512])
        nc.sync.dma_start(out=xpt_sb[:, 2:4, 0:512], in_=xpt_r[:, 2:4, 0:512])
        nc.sync.dma_start(out=xt_sb[:, :, 0:512], in_=xt_r[:, :, 0:512])
        nc.scalar.dma_start(out=wkv_sb[:, :, 0:128], in_=wkv_r[:, :, 0:128])
        nc.scalar.dma_start(out=wq_sb[:, :, 0:128], in_=wq_r[:, :, 0:128])
        nc.scalar.dma_start(out=wkv_sb[:, :, 128:inner],
                            in_=wkv_r[:, :, 128:inner])
        nc.scalar.dma_start(out=wq_sb[:, :, 128:inner],
                            in_=wq_r[:, :, 128:inner])
        nc.scalar.dma_start(out=bout_row[:, :], in_=bout_d.ap())
        nc.gpsimd.dma_start(out=wkv_sb[:, :, inner:2 * inner],
                            in_=wkv_r[:, :, inner:2 * inner])
        for c in range(1, nk // 512):
            nc.gpsimd.dma_start(out=xpt_sb[:, :, c * 512:(c + 1) * 512],
                                in_=xpt_r[:, :, c * 512:(c + 1) * 512])
        if nq > 512:
            nc.gpsimd.dma_start(out=xt_sb[:, :, 512:nq],
                                in_=xt_r[:, :, 512:nq])
        nc.gpsimd.dma_start(out=wout_sb[:, :, :], in_=wout_d.ap())
        # rows 64:128 of the last head pair, re-homed to partitions 0:64 so
        # the tail can finish pair npr-1 h1 straight from its tmp tile.
        woutl_sb = consts.tile([64, dim], CDT)
        nc.gpsimd.dma_start(out=woutl_sb[:, :],
                            in_=wout_d.ap()[64:128, npr - 1, :])
        nc.vector.memset(ones_sb[:, :], 1.0)
        # ones row at partition 64 for the K=1 broadcast matmul (1/l row
        # lives at partition 64, so lhsT/rhs share that offset).
        ones65 = consts.tile([VW, 128], CDT)
        nc.vector.memset(ones65[DH:VW, :], 1.0)

        for hh in range(h):  # ones columns of V
            nc.vector.memset(v_sb[:, :, hh * VW + DH:hh * VW + DH + 1], 1.0)

        for _rep in range(reps):
            rep_stack = contextlib.ExitStack()
            group_stack = contextlib.ExitStack()
            st_ps = group_stack.enter_context(
                tc.tile_pool(name="st_ps", bufs=2, space="PSUM"))
            pv_ps = group_stack.enter_context(
                tc.tile_pool(name="pv_ps", bufs=2, space="PSUM"))
            misc_ps = group_stack.enter_context(
                tc.tile_pool(name="misc_ps", bufs=2, space="PSUM"))
            pt_pool = rep_stack.enter_context(tc.tile_pool(name="pt", bufs=2))
            lr_pool = rep_stack.enter_context(tc.tile_pool(name="lr", bufs=4))
            fo_pool = rep_stack.enter_context(tc.tile_pool(name="fo", bufs=4))
            last_tmp = None

            # ---- projection / out-proj packet emitters -------------------
            def emit_proj(kind, a, c):
                ps = misc_ps.tile([128, 512], FDT, tag="mm")
                if kind == "kt":
                    for k in range(nkt):
                        nc.tensor.matmul(
                            ps[:, :], lhsT=wkv_sb[:, k, a * 128:(a + 1) * 128],
                            rhs=xpt_sb[:, k, c * 512:(c + 1) * 512],
                            start=(k == 0), stop=(k == nkt - 1))
                    nc.vector.tensor_copy(
                        out=kt_sb[:, a, c * 512:(c + 1) * 512], in_=ps[:, :])
                elif kind == "qt":
                    for k in range(nkt):
                        nc.tensor.matmul(
                            ps[:, :], lhsT=wq_sb[:, k, a * 128:(a + 1) * 128],
                            rhs=xt_sb[:, k, c * 512:(c + 1) * 512],
                            start=(k == 0), stop=(k == nkt - 1))
                    nc.vector.tensor_copy(
                        out=qt_sb[:, a, c * 512:(c + 1) * 512], in_=ps[:, :])
                else:  # v: natural layout, lhsT = xpt token-tile
                    for k in range(nkt):
                        nc.tensor.matmul(
                            ps[:, 0:inner],
                            lhsT=xpt_sb[:, k, a * 128:(a + 1) * 128],
                            rhs=wkv_sb[:, k, inner:2 * inner],
                            start=(k == 0), stop=(k == nkt - 1))
                    nc.vector.tensor_copy(
                        out=v_sb[:, a, :].rearrange(
                            "p (g x) -> p g x", x=VW)[:, :, 0:DH],
                        in_=ps[:, 0:inner].rearrange("p (g x) -> p g x", x=DH))

            def emit_op(c, t):  # out-proj row-tile t (queries t*128..+128)
                f = misc_ps.tile([128, 512], FDT, tag="mm")
                for g in range(npr):
                    nc.tensor.matmul(
                        f[:, 0:dim],
                        lhsT=aot2[:, g, t * 128:(t + 1) * 128],
                        rhs=wout_sb[:, g, :],
                        start=(g == 0), stop=False)
                nc.tensor.matmul(f[:, 0:dim], lhsT=ones_sb[:, 0:128],
                                 rhs=bout_row[:, :], start=False, stop=True)
                fo = fo_pool.tile([128, dim], FDT, tag="fo")
                nc.vector.tensor_copy(out=fo[:, :], in_=f[:, 0:dim])
                nc.sync.dma_start(
                    out=out_d.ap()[t * 128:(t + 1) * 128, :], in_=fo[:, :])

            def finish_group(pend, is_last):
                # 1/l broadcast via K=1 matmul (partition 64 -> rows 0:64),
                # PSUM->SBUF copy, normalize muls, h1 partition-shift DMA.
                p, c, pv0, pv1, r0, r1 = pend
                cs = slice(c * 512, (c + 1) * 512)
                rb0ps = misc_ps.tile([128, 512], FDT, tag="mm")
                nc.tensor.matmul(rb0ps[0:DH, :], lhsT=ones65[DH:VW, 0:DH],
                                 rhs=r0[DH:VW, :], start=True, stop=True)
                rb1ps = misc_ps.tile([128, 512], FDT, tag="mm")
                nc.tensor.matmul(rb1ps[0:DH, :], lhsT=ones65[DH:VW, 0:DH],
                                 rhs=r1[DH:VW, :], start=True, stop=True)
                rb0 = lr_pool.tile([DH, 512], FDT, tag="rb0")
                rb1 = lr_pool.tile([DH, 512], FDT, tag="rb1")
                nc.vector.tensor_copy(out=rb0[:, :], in_=rb0ps[0:DH, :])
                nc.vector.tensor_copy(out=rb1[:, :], in_=rb1ps[0:DH, :])
                tmp = lr_pool.tile([DH, 512], CDT, tag="tmp")
                if "nonorm" in stub:
                    nc.vector.tensor_copy(out=aot2[0:64, p, cs],
                                          in_=pv0[0:DH, :])
                    nc.vector.tensor_copy(out=tmp[:, :], in_=pv1[0:DH, :])
                else:
                    nc.vector.tensor_mul(aot2[0:64, p, cs],
                                         pv0[0:DH, :], rb0[:, :])
                    nc.vector.tensor_mul(tmp[:, :], pv1[0:DH, :], rb1[:, :])
                if not is_last:
                    nc.gpsimd.dma_start(out=aot2[64:128, p, cs], in_=tmp[:, :])
                return tmp

            # ---- schedule: prologue, then 8 fused groups -----------------
            # PE p-state warm-up: the tensor engine reaches full clock only
            # after ~3us of continuous execution. Run throwaway matmuls on
            # ones while the first input DMAs are in flight so the real
            # prologue starts at speed.
            wu = misc_ps.tile([128, 512], FDT, tag="mm")
            for i in range(3):
                nc.tensor.matmul(
                    wu[:, :], lhsT=ones_sb[:, 0:128],
                    rhs=ones_sb[:, :], start=True, stop=True)
            emit_proj("kt", 0, 0)
            emit_proj("qt", 0, 0)

            groups = [(p, c) for p in range(npr) for c in range(nqc)]
            spread = {g: [] for g in groups}
            pops = {g: {} for g in groups}

            def assign(g, j, pkt):
                pops[g][j] = len(spread[g])
                spread[g].append(pkt)

            for c in range(1, nk // 512):       # rest of kt(0): in group 0
                assign(groups[0], 3 * c - 2, ("kt", 0, c))
            for p in range(1, npr):             # pair p: one group ahead
                g = groups[2 * p - 1]
                for c in range(nk // 512):
                    assign(g, 2 * c + 1, ("kt", p, c))
                assign(g, 9, ("qt", p, 0))
            for p in range(npr):                # late-chunk qt: park it in
                for c in range(1, nqc):         # the act-bound group (p, 0)
                    if p == 0:
                        assign(groups[0], 10, ("qt", p, c))
                    else:
                        assign(groups[2 * p], 2, ("qt", p, c))
            for t in range(4 * (nqc - 1)):      # chunk-0 out-proj: last
                assign(groups[-1], 6 + 3 * t, ("op", t // 4, t))
            pending = None
            for gi, (p, c) in enumerate(groups):
                h0, h1 = 2 * p, 2 * p + 1
                pt = pt_pool.tile([128, nj, 1024], CDT, tag="pt")
                pv0 = pv1 = None
                queue = list(spread[(p, c)])
                pop_at = pops[(p, c)]

                for j in range(nj):
                    st = st_ps.tile([128, 1024], FDT, tag="st")
                    nc.tensor.matmul(
                        st[:, 0:512],
                        lhsT=kt_sb[0:64, p, j * 128:(j + 1) * 128],
                        rhs=qt_sb[0:64, p, c * 512:(c + 1) * 512],
                        start=True, stop=True)
                    nc.tensor.matmul(
                        st[:, 512:1024],
                        lhsT=kt_sb[64:128, p, j * 128:(j + 1) * 128],
                        rhs=qt_sb[64:128, p, c * 512:(c + 1) * 512],
                        start=True, stop=True)
                    if "noexp" in stub:
                        nc.vector.tensor_copy(out=pt[:, j, :], in_=st[:, :])
                    else:
                        nc.scalar.activation(out=pt[:, j, :], in_=st[:, :],
                                             func=Exp, scale=scale)
                    if (p, c) == (0, 0):
                        emit_proj("v", j, 0)
                    qi = pop_at.get(j)
                    if qi is not None and qi < len(queue):
                        kind, a, cc = queue[qi]
                        if kind == "op":
                            emit_op(a, cc)
                        else:
                            emit_proj(kind, a, cc)
                    if j > lag - 1:  # PV lags scores by `lag` key-tiles:
                        # deep enough that the previous group's ~6us 1/l
                        # DRAM-bounce chain clears before its PSUM pair is
                        # reused, shallow enough that the PV tail doesn't
                        # starve on the exp stream.
                        if j == lag:
                            pv0 = pv_ps.tile([VW, 512], FDT, tag="pv")
                            pv1 = pv_ps.tile([VW, 512], FDT, tag="pv")
                        nc.tensor.matmul(
                            pv0[:, :],
                            lhsT=v_sb[:, j - lag, h0 * VW:(h0 + 1) * VW],
                            rhs=pt[:, j - lag, 0:512],
                            start=(j == lag), stop=False)
                        nc.tensor.matmul(
                            pv1[:, :],
                            lhsT=v_sb[:, j - lag, h1 * VW:(h1 + 1) * VW],
                            rhs=pt[:, j - lag, 512:1024],
                            start=(j == lag), stop=False)
                for jj in range(nj - lag, nj):
                    nc.tensor.matmul(
                        pv0[:, :], lhsT=v_sb[:, jj, h0 * VW:(h0 + 1) * VW],
                        rhs=pt[:, jj, 0:512], start=False, stop=(jj == nj - 1))
                    nc.tensor.matmul(
                        pv1[:, :], lhsT=v_sb[:, jj, h1 * VW:(h1 + 1) * VW],
                        rhs=pt[:, jj, 512:1024], start=False,
                        stop=(jj == nj - 1))

                # softmax denominators: 1/l, then partition-broadcast
                # via a DRAM bounce (engine-free; the deep PV lag hides its
                # ~6us latency). The LAST group instead uses the low-latency
                # K=1-matmul broadcast in the tail.
                cs = slice(c * 512, (c + 1) * 512)
                if use_mm_chain and gi < len(groups) - 1:
                    r0 = lr_pool.tile([VW, 512], CDT, tag="r0b")
                    r1 = lr_pool.tile([VW, 512], CDT, tag="r1b")
                    with nc.allow_low_precision(reason="1/l bf16 bcast mm"):
                        nc.vector.reciprocal(out=r0[DH:VW, :],
                                             in_=pv0[DH:VW, :])
                        nc.vector.reciprocal(out=r1[DH:VW, :],
                                             in_=pv1[DH:VW, :])
                    finish_group((p, c, pv0, pv1, r0, r1), False)
                elif gi < len(groups) - 1:
                    ra = lr_pool.tile([VW, 512], FDT, tag="r0")
                    rc = lr_pool.tile([VW, 512], FDT, tag="r1")
                    nc.vector.reciprocal(out=ra[DH:VW, :], in_=pv0[DH:VW, :])
                    nc.vector.reciprocal(out=rc[DH:VW, :], in_=pv1[DH:VW, :])
                    idx = 2 * gi
                    nc.sync.dma_start(out=rsc_d.ap()[idx:idx + 1, :],
                                      in_=ra[DH:VW, :])
                    nc.sync.dma_start(out=rsc_d.ap()[idx + 1:idx + 2, :],
                                      in_=rc[DH:VW, :])
                    rb0 = lr_pool.tile([DH, 512], FDT, tag="rb0")
                    rb1 = lr_pool.tile([DH, 512], FDT, tag="rb1")
                    nc.gpsimd.dma_start(
                        out=rb0[:, :],
                        in_=rsc_d.ap()[idx:idx + 1, :].to_broadcast([DH, 512]))
                    nc.gpsimd.dma_start(
                        out=rb1[:, :],
                        in_=rsc_d.ap()[idx + 1:idx + 2, :].to_broadcast(
                            [DH, 512]))
                    tmp = lr_pool.tile([DH, 512], CDT, tag="tmp")
                    if "nonorm" in stub:
                        nc.vector.tensor_copy(out=aot2[0:64, p, cs],
                                              in_=pv0[0:DH, :])
                        nc.vector.tensor_copy(out=tmp[:, :], in_=pv1[0:DH, :])
                    else:
                        nc.vector.tensor_mul(aot2[0:64, p, cs],
                                             pv0[0:DH, :], rb0[:, :])
                        nc.vector.tensor_mul(tmp[:, :], pv1[0:DH, :],
                                             rb1[:, :])
                    nc.gpsimd.dma_start(out=aot2[64:128, p, cs], in_=tmp[:, :])
                else:
                    r0 = lr_pool.tile([VW, 512], CDT, tag="r0b")
                    r1 = lr_pool.tile([VW, 512], CDT, tag="r1b")
                    with nc.allow_low_precision(reason="1/l in bf16 feeds a "
                                                "bf16 matmul broadcast"):
                        nc.vector.reciprocal(out=r0[DH:VW, :],
                                             in_=pv0[DH:VW, :])
                        nc.vector.reciprocal(out=r1[DH:VW, :],
                                             in_=pv1[DH:VW, :])
                    pending = (p, c, pv0, pv1, r0, r1)

            # tail: out-proj of the last chunk. The last head-pair's 1/l
            # chain (recip -> bcast -> mul -> partition-shift DMA) has
            # ~4.5us latency; cover it with the pair-0..2 accumulation
            # matmuls (ready much earlier), then finish with pair 3.
            last_tmp = finish_group(pending, True)
            group_stack.close()
            tail_ps = rep_stack.enter_context(
                tc.tile_pool(name="tail_ps", bufs=4, space="PSUM"))
            t_range = list(range(4 * (nqc - 1), 4 * nqc))
            fts = []
            for t in t_range:
                f = tail_ps.tile([128, 512], FDT, tag="tf")
                for g in range(npr - 1):
                    nc.tensor.matmul(
                        f[:, 0:dim],
                        lhsT=aot2[:, g, t * 128:(t + 1) * 128],
                        rhs=wout_sb[:, g, :],
                        start=(g == 0), stop=False)
                fts.append(f)
            for i, (f, t) in enumerate(zip(fts, t_range)):
                tco = t * 128 - (nqc - 1) * 512
                nc.tensor.matmul(
                    f[:, 0:dim],
                    lhsT=aot2[0:64, npr - 1, t * 128:(t + 1) * 128],
                    rhs=wout_sb[0:64, npr - 1, :],
                    start=False, stop=False)
                nc.tensor.matmul(
                    f[:, 0:dim],
                    lhsT=last_tmp[:, tco:tco + 128],
                    rhs=woutl_sb[:, :],
                    start=False, stop=False)
                nc.tensor.matmul(f[:, 0:dim], lhsT=ones_sb[:, 0:128],
                                 rhs=bout_row[:, :], start=False, stop=True)
                fo = fo_pool.tile([128, dim], FDT, tag="fo")
                if i % 2 == 0:
                    nc.vector.tensor_copy(out=fo[:, :], in_=f[:, 0:dim])
                    nc.sync.dma_start(
                        out=out_d.ap()[t * 128:(t + 1) * 128, :], in_=fo[:, :])
                else:
                    # Act is idle at the tail; Copy shares the Exp table
                    nc.scalar.activation(out=fo[:, :], in_=f[:, 0:dim],
                                         func=mybir.ActivationFunctionType.Copy)
                    nc.scalar.dma_start(
                        out=out_d.ap()[t * 128:(t + 1) * 128, :], in_=fo[:, :])
            rep_stack.close()

    if compile_module:
        nc.compile()
    return nc


def host_inputs(x, x_prev, w_q, w_kv, w_out, b_out, ncores=NCORES):
    """Shard + lay out the full inputs into per-core input maps."""
    bf16 = ml_dtypes.bfloat16
    b, n, dim = x.shape
    inner = w_q.shape[1]
    h = inner // DH
    npr = h // 2
    nq = (b * n) // ncores
    halves = ncores // b
    wq = np.ascontiguousarray(w_q).astype(bf16)
    wkv = np.ascontiguousarray(w_kv).astype(bf16)
    wout = np.ascontiguousarray(
        w_out.reshape(npr, 128, dim).transpose(1, 0, 2)).astype(bf16)
    bout = np.ascontiguousarray(b_out).reshape(1, dim).astype(bf16)
    in_maps = []
    for c in range(ncores):
        bb, half = c // halves, c % halves
        xt = np.ascontiguousarray(
            x[bb, half * nq:(half + 1) * nq, :].T).astype(bf16)
        xpt = np.ascontiguousarray(x_prev[bb].T).astype(bf16)
        in_maps.append(dict(xt=xt, xpt=xpt, wq=wq, wkv=wkv, wout=wout,
                            bout=bout))
    return in_maps


def _get_module():
    global _BUILT
    if _BUILT is None:
        _BUILT = build_module()
    return _BUILT


def kernel(x, x_prev, w_q, w_kv, w_out, b_out):
    from concourse.bass_utils import run_bass_kernel_spmd

    nc = _get_module()
    in_maps = host_inputs(x, x_prev, w_q, w_kv, w_out, b_out)
    res = run_bass_kernel_spmd(nc, in_maps, core_ids=list(range(NCORES)))

    nq = N // 2
    out = np.empty((B, N, DIM), np.float32)
    for c in range(NCORES):
        b, half = c // 2, c % 2
        out[b, half * nq:(half + 1) * nq, :] = res.results[c]["out"]
    return out
